# revision 19
# baseline (speedup 1.0000x reference)
"""Trainium2 Bass kernel: AtomEmbeddingAndSumLastLayer (segment_reduce).

Computes: out = normalize(relu(segment_sum(relu(x @ W.T + b), batch)))
  x [1M, 118] f32, W [64, 118], b [64], batch [1M] sorted int in [0, 4096).

Strategy (8 NeuronCores, no collectives needed):
  - Atoms are cut at segment-aligned boundaries on the host so core c owns
    exactly segments [512c, 512(c+1)). Per-core outputs concatenate to the
    full [4096, 64] result.
  - Host pre-transposes x to xT [119, A] bf16 (row 118 = ones, so the bias
    folds into the matmul as an extra contraction row) and computes
    window-local segment ids per atom.
  - On device, per 128-atom tile:
      h_psum[128, 64] = xT_tile.T @ WT           (TensorE, f32 psum)
      h_sb = relu(h_psum) -> bf16                (ScalarE/VectorE)
      oh[128, Gw] = (iota == seg_local)          (VectorE tensor_scalar)
      s_psum[Gw, 64] += oh.T @ h_sb              (TensorE, accumulated over
                                                  all tiles of the window)
    Epilogue per window: rowwise max, reciprocal, scale, DMA out.
"""

import os
import sys
import numpy as np

sys.path.insert(0, "/opt/trn_rl_repo")

import ml_dtypes  # noqa: E402

N_ATOMS = 1_000_000
D_IN = 118
K_DIM = 128  # 118 features + ones-row (bias) at 118, zero-padded to 128
ONES_ROW = D_IN  # row index carrying the bias ones
D_OUT = 64
NUM_SEG = 4096
N_CORES = 8
SEGS_PER_CORE = NUM_SEG // N_CORES  # 512
G_W = 16  # segments per PSUM window
WINDOWS = SEGS_PER_CORE // G_W  # 16
P = 128
CHUNK = 16  # tiles per compute chunk (16 * 64 f32 = two psum banks)
EPI_GROUP = 4  # windows normalized/written per epilogue batch
PAD_ID = 200.0  # local seg id for padding atoms; never matches iota [0, G_W)

BF16 = ml_dtypes.bfloat16
FP8 = ml_dtypes.float8_e4m3

_CACHE = {}


def _build_graph(t_w: int, postprocess: bool = True):
    """Build the SPMD Bass graph for one core. t_w = tiles per window."""
    import concourse.bass as bass
    import concourse.tile as tile
    from concourse import mybir
    from contextlib import ExitStack

    n_tiles = WINDOWS * t_w
    a_cols = n_tiles * P  # padded atoms per core

    nc = bass.Bass(target_bir_lowering=False)

    xt = nc.declare_dram_parameter("xt", [K_DIM, a_cols], mybir.dt.float8e4, False)
    seg = nc.declare_dram_parameter("seg", [P, n_tiles], mybir.dt.bfloat16, False)
    wt = nc.declare_dram_parameter("wt", [K_DIM, D_OUT], mybir.dt.bfloat16, False)
    iota = nc.declare_dram_parameter("iota", [P, G_W], mybir.dt.bfloat16, False)
    out = nc.declare_dram_parameter(
        "out", [SEGS_PER_CORE, D_OUT], mybir.dt.float32, True
    )

    with ExitStack() as ctx:
        tc = ctx.enter_context(tile.TileContext(nc))
        consts = ctx.enter_context(tc.tile_pool(name="consts", bufs=1))
        xpool = ctx.enter_context(tc.tile_pool(name="xp", bufs=6))
        hpool = ctx.enter_context(tc.tile_pool(name="hp", bufs=4))
        ohpool = ctx.enter_context(tc.tile_pool(name="ohp", bufs=3))
        psum_h = ctx.enter_context(tc.tile_pool(name="psh", bufs=3, space="PSUM"))
        psum_s = ctx.enter_context(tc.tile_pool(name="pss", bufs=2, space="PSUM"))
        epi = ctx.enter_context(tc.tile_pool(name="epi", bufs=2))

        wt_sb = consts.tile([K_DIM, D_OUT], mybir.dt.bfloat16)
        nc.sync.dma_start(out=wt_sb[:], in_=wt[:, :])
        iota_sb = consts.tile([P, G_W], mybir.dt.bfloat16)
        nc.sync.dma_start(out=iota_sb[:], in_=iota[:, :])
        seg_sb = consts.tile([P, n_tiles], mybir.dt.bfloat16)
        nc.sync.dma_start(out=seg_sb[:], in_=seg[:, :])

        # "touch" the consts on VectorE once so later ops don't each carry
        # multiple DMA-lane semaphore waits (walrus wait-slot limit).
        dummy_a = consts.tile([P, 1], mybir.dt.bfloat16)
        nc.vector.tensor_copy(out=dummy_a[:], in_=iota_sb[:, :1])
        dummy_b = consts.tile([P, 1], mybir.dt.bfloat16)
        nc.vector.tensor_copy(out=dummy_b[:], in_=seg_sb[:, :1])
        dummy_c = consts.tile([K_DIM, 1], mybir.dt.bfloat16)
        nc.vector.tensor_copy(out=dummy_c[:], in_=wt_sb[:, :1])
        # prewarm ScalarE's activation table (one-time ~2.7us load) during
        # the initial x DMA instead of blocking the first relu
        dummy_d = consts.tile([P, 1], mybir.dt.bfloat16)
        nc.scalar.activation(
            out=dummy_d[:], in_=dummy_a[:],
            func=mybir.ActivationFunctionType.Relu,
        )

        n_chunks = t_w // CHUNK
        for w in range(WINDOWS):
            n_pieces = 4 if w == 0 else 2
            piece = t_w * P // n_pieces
            x_pieces = []
            for pi in range(n_pieces):
                xp_t = xpool.tile([K_DIM, piece], mybir.dt.float8e4,
                                  tag=f"xh{n_pieces}")
                p0 = w * t_w * P + pi * piece
                nc.sync.dma_start(out=xp_t[:], in_=xt[:, p0 : p0 + piece])
                x_pieces.append(xp_t)
            s_ps = psum_s.tile([G_W, D_OUT], mybir.dt.float32)
            # build the whole window's one-hot in one DVE op:
            # oh[p, t, g] = (iota[p, g] == seg[p, w*t_w + t])
            oh_win = ohpool.tile([P, t_w * G_W], mybir.dt.bfloat16)
            iota_ap = iota_sb[:]
            in0 = bass.AP(
                tensor=iota_ap.tensor, offset=iota_ap.offset,
                ap=[iota_ap.ap[0], [0, t_w], iota_ap.ap[1]],
            )
            seg_sl = seg_sb[:, w * t_w : (w + 1) * t_w]
            in1 = bass.AP(
                tensor=seg_sl.tensor, offset=seg_sl.offset,
                ap=[seg_sl.ap[0], seg_sl.ap[1], [0, G_W]],
            )
            nc.vector.tensor_tensor(
                out=oh_win[:].rearrange("p (t g) -> p t g", g=G_W),
                in0=in0, in1=in1, op=mybir.AluOpType.is_equal,
            )
            for chv in range(n_chunks):
                h_ps = psum_h.tile([P, CHUNK * D_OUT], mybir.dt.float32)
                for i in range(CHUNK):
                    t = chv * CHUNK + i
                    pi = (t * P) // piece
                    toff = pi * piece
                    nc.tensor.matmul(
                        out=h_ps[:, i * D_OUT : (i + 1) * D_OUT],
                        lhsT=x_pieces[pi][:, t * P - toff : (t + 1) * P - toff],
                        rhs=wt_sb[:],
                        start=True,
                        stop=True,
                    )
                h_sb = hpool.tile([P, CHUNK * D_OUT], mybir.dt.bfloat16)
                if (w * n_chunks + chv) % 4 == 3:
                    nc.vector.tensor_scalar_max(
                        out=h_sb[:], in0=h_ps[:], scalar1=0.0
                    )
                else:
                    nc.scalar.activation(
                        out=h_sb[:],
                        in_=h_ps[:],
                        func=mybir.ActivationFunctionType.Relu,
                    )
                for i in range(CHUNK):
                    t = chv * CHUNK + i
                    nc.tensor.matmul(
                        out=s_ps[:],
                        lhsT=oh_win[:, t * G_W : (t + 1) * G_W],
                        rhs=h_sb[:, i * D_OUT : (i + 1) * D_OUT],
                        start=(chv == 0 and i == 0),
                        stop=(chv == n_chunks - 1 and i == CHUNK - 1),
                    )
            # stage this window's sums along the free axis; one epilogue
            # (reduce/recip/scale/DMA) per EPI_GROUP windows
            gidx = w % EPI_GROUP
            if gidx == 0:
                s_stage = epi.tile([G_W, EPI_GROUP * D_OUT], mybir.dt.float32)
            nc.vector.tensor_copy(
                out=s_stage[:, gidx * D_OUT : (gidx + 1) * D_OUT], in_=s_ps[:]
            )
            if gidx == EPI_GROUP - 1:
                st3 = s_stage[:].rearrange("p (g o) -> p g o", o=D_OUT)
                mx = epi.tile([G_W, EPI_GROUP], mybir.dt.float32)
                nc.vector.tensor_reduce(
                    out=mx[:], in_=st3, axis=mybir.AxisListType.X,
                    op=mybir.AluOpType.max,
                )
                rc = epi.tile([G_W, EPI_GROUP], mybir.dt.float32)
                nc.vector.reciprocal(out=rc[:], in_=mx[:])
                o_sb = epi.tile([G_W, EPI_GROUP * D_OUT], mybir.dt.float32)
                rc_ap = rc[:]
                rc_b = bass.AP(
                    tensor=rc_ap.tensor, offset=rc_ap.offset,
                    ap=[rc_ap.ap[0], rc_ap.ap[1], [0, D_OUT]],
                )
                nc.vector.tensor_tensor(
                    out=o_sb[:].rearrange("p (g o) -> p g o", o=D_OUT),
                    in0=st3, in1=rc_b, op=mybir.AluOpType.mult,
                )
                w0 = (w - EPI_GROUP + 1) * G_W
                out_r = out[w0 : w0 + EPI_GROUP * G_W, :].rearrange(
                    "(g p) o -> p g o", p=G_W
                )
                nc.sync.dma_start(
                    out=out_r,
                    in_=o_sb[:].rearrange("p (g o) -> p g o", o=D_OUT),
                )

    if postprocess:
        _split_multi_waits(nc)
    return nc


def _split_multi_waits(nc):
    """walrus allows a single embedded sync wait per compute instruction.
    Move extra waits onto same-engine NoOps inserted just before."""
    from concourse import mybir

    n = 0
    for f in nc.m.functions:
        for blk in f.blocks:
            new_insts = []
            for inst in blk.instructions:
                si = getattr(inst, "sync_info", None)
                if si is not None and si.on_wait and len(si.on_wait) > 1:
                    extras, keep = si.on_wait[:-1], si.on_wait[-1:]
                    for wsub in extras:
                        nop = mybir.InstNoOp(
                            name=f"{inst.name}_waitnop{n}",
                            sync_info=mybir.SyncInfo(on_wait=[wsub], on_update=[]),
                            bass_nofuse=True,
                            engine=inst.engine,
                        )
                        n += 1
                        new_insts.append(nop)
                    si.on_wait = keep
                new_insts.append(inst)
            blk.instructions[:] = new_insts


def _strip_pe_self_waits(nc):
    """Remove PE-semaphore waits from PE-engine instructions.

    TensorE executes matmuls strictly in order, so a PE instruction's wait
    on the PE completion semaphore is always satisfied by program order.
    Tile emits them conservatively for PSUM zero-region reuse, but walrus
    allows only one embedded sync wait per Matmult instruction.
    """
    from concourse import mybir

    for f in nc.m.functions:
        for blk in f.blocks:
            for inst in blk.instructions:
                if getattr(inst, "engine", None) != mybir.EngineType.PE:
                    continue
                si = getattr(inst, "sync_info", None)
                if si is None or not si.on_wait:
                    continue
                kept = [w for w in si.on_wait if not str(w.ant_name).startswith("PE")]
                if len(kept) != len(si.on_wait):
                    si.on_wait = kept


def _prepare_inputs(x, w_mat, b, batch):
    """Host-side sharding/layout. Returns (in_maps, t_w)."""
    x = np.asarray(x, dtype=np.float32)
    w_mat = np.asarray(w_mat, dtype=np.float32)
    b = np.asarray(b, dtype=np.float32)
    batch = np.asarray(batch).astype(np.int64)
    n = x.shape[0]

    # window boundaries: atoms for window j (global) are [wb[j], wb[j+1])
    n_windows_total = NUM_SEG // G_W  # 128
    wb = np.searchsorted(batch, np.arange(0, NUM_SEG + 1, G_W))
    counts = np.diff(wb)  # atoms per window
    t_w = int(np.ceil(counts.max() / P))
    t_w = ((t_w + CHUNK - 1) // CHUNK) * CHUNK  # round to CHUNK multiple

    n_tiles = WINDOWS * t_w
    a_cols = n_tiles * P

    wt = np.zeros((K_DIM, D_OUT), dtype=BF16)
    wt[:D_IN] = w_mat.T.astype(BF16)
    wt[ONES_ROW] = b.astype(BF16)
    iota = np.broadcast_to(
        np.arange(G_W, dtype=np.float32), (P, G_W)
    ).astype(BF16)

    xb = x.astype(FP8)  # one bulk cast
    in_maps = []
    for c in range(N_CORES):
        xt_c = np.zeros((K_DIM, a_cols), dtype=FP8)
        seg_c = np.full((n_tiles, P), PAD_ID, dtype=np.float32)
        for w in range(WINDOWS):
            gw = c * WINDOWS + w  # global window index
            a0, a1 = wb[gw], wb[gw + 1]
            cnt = a1 - a0
            off = w * t_w * P
            xt_c[:D_IN, off : off + cnt] = xb[a0:a1].T
            xt_c[ONES_ROW, off : off + cnt] = 1.0
            loc = (batch[a0:a1] - gw * G_W).astype(np.float32)
            seg_c.reshape(-1)[off : off + cnt] = loc
        seg_c = np.ascontiguousarray(seg_c.reshape(n_tiles, P).T).astype(BF16)
        in_maps.append({"xt": xt_c, "seg": seg_c, "wt": wt, "iota": iota})
    return in_maps, t_w


def _install_ntff_hook_shim():
    """The trimmed container's antenv lacks axon_hooks; recreate it so
    run_bass_kernel_spmd(trace=True) can profile via the axon .so."""
    import types

    if "antenv.axon_hooks" in sys.modules:
        return
    try:
        from trn_agent_boot.trn_boot import _ntff_profile_via_ctypes

        hook = _ntff_profile_via_ctypes("/opt/axon/libaxon_pjrt.so")
    except Exception:
        hook = None
    mod = types.ModuleType("antenv.axon_hooks")
    mod._hook = hook
    mod.get_axon_ntff_profile_hook = lambda: mod._hook
    mod.set_axon_ntff_profile_hook = lambda h: setattr(mod, "_hook", h)
    sys.modules["antenv.axon_hooks"] = mod


def kernel(x, W, b, batch, num_segments):
    from concourse.bass_utils import run_bass_kernel_spmd

    assert int(num_segments) == NUM_SEG
    in_maps, t_w = _prepare_inputs(x, W, b, batch)

    if t_w not in _CACHE:
        _CACHE[t_w] = _build_graph(t_w)
    nc = _CACHE[t_w]

    trace = bool(int(os.environ.get("KERNEL_TRACE", "0")))
    if trace:
        _install_ntff_hook_shim()
    res = run_bass_kernel_spmd(
        nc, in_maps, core_ids=list(range(N_CORES)), trace=trace
    )
    kernel.last_result = res
    out = np.concatenate([r["out"] for r in res.results], axis=0)
    return out.astype(np.float32)


kernel.last_result = None
